# revision 2
# baseline (speedup 1.0000x reference)
"""Causal multi-head attention block (B=8, T=2048, C=768, H=8) on 8 trn2 cores.

Sharding: data-parallel over batch — one batch element per NeuronCore, weights
replicated, no collectives.

On top of v2 (all-bf16 SBUF-resident, no DRAM scratch, paired-j-tile attention
with fed-in projection work and a double-buffered softmax epilogue), v4 packs
the Q/K projections into 12 M=128 stripe groups (instead of 16 M=96 head
groups): the projection output is evicted into a packed staging tile and
repacked into head-aligned [96, T] q/k tiles by sb->sb DMA on the otherwise
idle sync queue. That removes a quarter of the projection matmul cycles and
96 weight-load serializations.

Phases per core:
  A: x / w_attn / w_proj PE-transposed into resident bf16 tiles, x in 256-row
     chunks fused into head 0's blocks.
  B: per head: causal attention in S^T layout over 512-wide i-blocks and
     j-tile pairs (one exp per pair; PV runs one pair behind its S/exp so the
     exp latency hides behind the next pair's S plus fed projection items;
     diagonal masks on Pool). The denominator accumulates via a ones column
     in PSUM row 64; the epilogue reciprocal (approx DVE op), PE broadcast,
     DVE normalize, and a sb->sb DMA pack into 128-row O^T stripes.
  C: out = O^T.T @ w_proj^T + b_proj over packed stripes, fed into head 7.
"""

import math
import os
import sys
from contextlib import ExitStack

for _p in ("/opt/trn_rl_repo", "/root/.axon_site/_ro/trn_rl_repo"):
    if os.path.isdir(_p) and _p not in sys.path:
        sys.path.append(_p)

import numpy as np
import ml_dtypes

import concourse.bass as bass  # noqa: F401
from concourse import bacc
import concourse.mybir as mybir
import concourse.tile as tile
from concourse.bass_utils import run_bass_kernel_spmd

F32 = mybir.dt.float32
F32R = mybir.dt.float32r
BF16 = mybir.dt.bfloat16
EXP = mybir.ActivationFunctionType.Exp
ADD = mybir.AluOpType.add
MULT = mybir.AluOpType.mult

B, T, C, H, HS = 8, 2048, 768, 8, 96
KT = C // 128         # 6 contraction tiles of 128
TT = T // 128         # 16 t-tiles of 128
NB = T // 512         # 4 i-blocks of 512
NCORES = 8


def build_nc():
    nc = bacc.Bacc()
    x_b = nc.dram_tensor("x_b", [T, C], F32R, kind="ExternalInput")
    wat = nc.dram_tensor("wat", [3 * C, C], F32R, kind="ExternalInput")
    wp = nc.dram_tensor("wp", [C, C], F32R, kind="ExternalInput")
    ident = nc.dram_tensor("ident", [128, 128], F32R, kind="ExternalInput")
    mk = nc.dram_tensor("mk", [128, 128], BF16, kind="ExternalInput")
    mkz = nc.dram_tensor("mkz", [128, 256], BF16, kind="ExternalInput")
    onesc = nc.dram_tensor("onesc", [128, 128], BF16, kind="ExternalInput")
    bqkp = nc.dram_tensor("bqkp", [128, 12], F32, kind="ExternalInput")
    bv = nc.dram_tensor("bv", [128, C], F32, kind="ExternalInput")
    bo = nc.dram_tensor("bo", [128, C], F32, kind="ExternalInput")
    out = nc.dram_tensor("out", [T, C], F32, kind="ExternalOutput")

    x_r = x_b.rearrange("(a p) c -> p a c", p=128)     # [128, 16, C]
    wat_r = wat.rearrange("(a p) c -> p a c", p=128)   # [128, 18, C]
    wp_r = wp.rearrange("(a p) c -> p a c", p=128)     # [128, 6, C]
    out_r = out.rearrange("(a p) c -> p a c", p=128)   # [128, 16, C]

    with tile.TileContext(nc) as tc, ExitStack() as ctx:
        # ---------------- pools ----------------
        consts = ctx.enter_context(tc.tile_pool(name="consts", bufs=1))
        id_sb = consts.tile([128, 128], F32R, tag="id")
        mk_sb = consts.tile([128, 128], BF16, tag="mk")
        mkz_sb = consts.tile([128, 256], BF16, tag="mkz")
        onesc_sb = consts.tile([128, 128], BF16, tag="onesc")
        bqkp_sb = consts.tile([128, 12], F32, tag="bqkp")
        bv_sb = consts.tile([128, C], F32, tag="bv")
        bo_sb = consts.tile([128, C], F32, tag="bo")
        one_bf = consts.tile([128, 1], BF16, tag="onebf")
        nc.vector.memset(one_bf[:], 1.0)

        persist = ctx.enter_context(tc.tile_pool(name="persist", bufs=1))
        xT = persist.tile([128, KT, T], BF16, tag="xT")        # x^T
        waT = persist.tile([128, KT, 3 * C], BF16, tag="waT")  # w_attn^T
        wpT = persist.tile([128, KT, C], BF16, tag="wpT")      # w_proj^T
        oTp = persist.tile([128, KT, T], BF16, tag="oTp")      # packed O^T

        pa_x = ctx.enter_context(tc.tile_pool(name="pa_x", bufs=2))
        pa_w = ctx.enter_context(tc.tile_pool(name="pa_w", bufs=3))
        pa_wp = ctx.enter_context(tc.tile_pool(name="pa_wp", bufs=2))
        vsbp = ctx.enter_context(tc.tile_pool(name="vsb", bufs=2))
        qkp = ctx.enter_context(tc.tile_pool(name="qk", bufs=6))
        stgp = ctx.enter_context(tc.tile_pool(name="stg", bufs=3))
        p2p = ctx.enter_context(tc.tile_pool(name="p2", bufs=2))
        epp = ctx.enter_context(tc.tile_pool(name="ep", bufs=2))
        ocp = ctx.enter_context(tc.tile_pool(name="oc", bufs=2))
        # PSUM: pj(2 bufs x 1 bank) + S-pair(2 bufs x 2 banks) + O(2 x 1) = 8
        pjps = ctx.enter_context(tc.tile_pool(name="pj", bufs=2, space="PSUM"))
        sps = ctx.enter_context(tc.tile_pool(name="sp", bufs=2, space="PSUM"))
        ops = ctx.enter_context(tc.tile_pool(name="op", bufs=2, space="PSUM"))

        # ---------------- DMA staging ----------------
        wch = {}

        def issue_w(s0, n=2):
            t_ = pa_w.tile([128, 2, C], F32R, tag="wch", name=f"w{s0}")
            nc.sync.dma_start(t_[:, 0:n, :], wat_r[:, s0:s0 + n, :])
            wch[s0] = (t_, n)

        nc.sync.dma_start(id_sb[:], ident[:, :])
        # x in 8 chunks of 2 t-tiles (first chunk first so transposes start
        # as early as possible)
        xch = [pa_x.tile([128, 2, C], F32R, tag="xch", name=f"xch{c2}")
               for c2 in range(8)]
        nc.sync.dma_start(xch[0][:], x_r[:, 0:2, :])
        nc.sync.dma_start(xch[1][:], x_r[:, 2:4, :])
        issue_w(0)       # q stripes 0,1 (heads 0-2)
        issue_w(6)       # k stripes 6,7 (heads 0-2)
        issue_w(12)      # v cols 0-255 (head pair 0)
        nc.sync.dma_start(mk_sb[:], mk[:, :])
        nc.sync.dma_start(mkz_sb[:], mkz[:, :])
        nc.sync.dma_start(onesc_sb[:], onesc[:, :])
        nc.sync.dma_start(bqkp_sb[:], bqkp[:, :])
        nc.sync.dma_start(bv_sb[:], bv[:, :])
        nc.sync.dma_start(bo_sb[:], bo[:, :])
        for c2 in range(2, 8):
            nc.sync.dma_start(xch[c2][:], x_r[:, 2 * c2:2 * c2 + 2, :])

        # ---------------- helpers ----------------
        def tr_group(src, idxs, kc, dst):
            n = len(idxs)
            psb = pjps.tile([128, 512], F32R, tag="pj", name="trps")
            for i, sl in enumerate(idxs):
                nc.tensor.matmul(psb[:, i * 128:(i + 1) * 128],
                                 src[:, sl, kc * 128:(kc + 1) * 128],
                                 id_sb[:], is_transpose=True,
                                 start=(i == 0), stop=(i == n - 1))
            nc.any.tensor_copy(dst, psb[:, 0:n * 128])

        def transpose_w(s0):
            t_, n = wch[s0]
            for kc in range(KT):
                tr_group(t_, range(n), kc,
                         waT[:, kc, s0 * 128:(s0 + n) * 128])

        def transpose_x(c2):
            for kc in range(KT):
                tr_group(xch[c2], range(2), kc,
                         xT[:, kc, c2 * 256:(c2 + 1) * 256])

        vtile = {}
        vstate = [None]

        def new_v():
            # V head columns at slots 0-63 / 96-127, ones at 64-95: the
            # denominator accumulates in PSUM row 64 and O in rows 0-63 /
            # 96-127 (>32-partition APs must start at partition 0; bases
            # 64/96 are legal for <=32-partition APs and 1-row operands).
            V = vsbp.tile([128, TT, 2, 128], BF16, tag="V", name="V")
            nc.vector.tensor_copy(
                V[:, :, :, 64:96],
                one_bf[:].to_broadcast([128, TT, 2, 32]))
            vstate[0] = V
            vtile[len(vtile)] = V

        def v_proj(pr, tts):
            V = vtile[pr]
            c0 = 2 * C + 2 * HS * pr
            for tt in tts:
                vps = pjps.tile([128, 512], F32, tag="pj", name="vps")
                for kc in range(KT):
                    nc.tensor.matmul(vps[:, 0:2 * HS],
                                     xT[:, kc, tt * 128:(tt + 1) * 128],
                                     waT[:, kc, c0:c0 + 2 * HS],
                                     start=(kc == 0), stop=(kc == KT - 1))
                vr = vps[:, 0:2 * HS].rearrange("p (h d) -> p h d", d=HS)
                br = (bv_sb[:, 2 * HS * pr:2 * HS * (pr + 1)]
                      .rearrange("p (h d) -> p h d", d=HS))
                nc.vector.tensor_tensor(V[:, tt, :, 0:64],
                                        vr[:, :, 0:64], br[:, :, 0:64], ADD)
                nc.vector.tensor_tensor(V[:, tt, :, 96:128],
                                        vr[:, :, 64:HS], br[:, :, 64:HS], ADD)

        # --- packed Q/K projection: stripe j = packed rows 128j..128j+127
        # of [q rows 0-767 | k rows 768-1535]; lhsT M=128 spans head
        # boundaries; repack into head-aligned q/k tiles via sync-queue DMA.
        qkt = {}     # head -> [qT, kT] tiles [96, T] bf16

        def new_qk(h):
            qkt[h] = [qkp.tile([HS, T], BF16, tag="qk", name=f"qk{h}_{i}")
                      for i in range(2)]

        def qk_chunk(j, tc4):
            """Stripe j projection for t-chunk tc4: 6-matmul chain, bias-add
            evict into a small staging tile, then sync-queue DMA repack into
            the head-aligned q/k tiles (a stripe spans two heads)."""
            st = stgp.tile([128, 512], BF16, tag="stg", name="stg")
            pj = pjps.tile([128, 512], F32, tag="pj", name="qkps")
            for kc in range(KT):
                nc.tensor.matmul(
                    pj[:, 0:512],
                    waT[:, kc, 128 * j:128 * (j + 1)],
                    xT[:, kc, tc4 * 512:(tc4 + 1) * 512],
                    start=(kc == 0), stop=(kc == KT - 1))
            nc.vector.tensor_tensor(
                st[:, :], pj[:, 0:512],
                bqkp_sb[:, j:j + 1].to_broadcast([128, 512]), ADD)
            m = 0 if j < 6 else 1
            rowbase = 128 * (j % 6)
            lo, hi = tc4 * 512, (tc4 + 1) * 512
            r = rowbase
            while r < rowbase + 128:
                hd = r // HS
                rend = min((hd + 1) * HS, rowbase + 128)
                nc.sync.dma_start(
                    qkt[hd][m][r - HS * hd:rend - HS * hd, lo:hi],
                    st[r - rowbase:rend - rowbase, :])
                r = rend

        epi_carry = [None]   # deferred epilogue tail of the previous block

        def flush_epi():
            if epi_carry[0] is not None:
                epi_carry[0]()
                epi_carry[0] = None

        def attn_block(h, hh, b, qT, kT_, V, feed=None, early_flush=False):
            """Causal attention for head h, i-block b; PV lags one pair; the
            epilogue's PE part (denominator broadcast) and normalize/pack run
            deferred, early in the NEXT block, so the PE never waits on the
            copy->reciprocal DVE chain at a block boundary."""
            ib = 512 * b
            O_ps = ops.tile([128, 512], F32, tag="O", name="Ops")
            npair = 2 * b + 2

            def emit_pv(P2, p, de):
                for e in range(2):
                    jt = 2 * p + e
                    nc.tensor.matmul(O_ps[0:128, de:512],
                                     V[:, jt, hh, :],
                                     P2[:, e, de:512],
                                     start=(p == 0 and e == 0),
                                     stop=(p == npair - 1 and e == 1))

            pend = []
            for p in range(npair):
                de = 0 if p <= 2 * b else 256
                S2 = sps.tile([128, 2, 512], F32, tag="S", name="S2")
                for e in range(2):
                    jt = 2 * p + e
                    nc.tensor.matmul(S2[:, e, de:512],
                                     kT_[0:HS, jt * 128:(jt + 1) * 128],
                                     qT[0:HS, ib + de:ib + 512],
                                     start=True, stop=True)
                P2 = p2p.tile([128, 2, 512], BF16, tag="P", name="P2")
                nc.scalar.activation(P2[:, :, de:512], S2[:, :, de:512], EXP)
                if p >= 2 * b:  # diagonal pair: causal masks
                    nc.gpsimd.tensor_tensor(P2[:, 0, de:de + 128],
                                            P2[:, 0, de:de + 128],
                                            mk_sb[:], MULT)
                    nc.gpsimd.tensor_tensor(P2[:, 1, de:de + 256],
                                            P2[:, 1, de:de + 256],
                                            mkz_sb[:], MULT)
                pend.append((P2, p, de))
                if p == 0 and early_flush:
                    flush_epi()
                if feed:
                    for _ in range(min(3, len(feed))):
                        feed.pop(0)()
                if p == 0:
                    flush_epi()
                if len(pend) > 1:
                    emit_pv(*pend.pop(0))
            while pend:
                emit_pv(*pend.pop(0))

            # epilogue, immediate DVE part
            lt = epp.tile([128, 512], F32, tag="lt", name="lt")
            nc.vector.tensor_copy(lt[:], O_ps[0:128, :])
            lr2 = epp.tile([65, 512], F32, tag="lr2", name="lr2", bufs=1)
            nc.vector.reciprocal_approx_fast(lr2[0:65, :], lt[0:65, :])
            Oe = epp.tile([128, 512], BF16, tag="Oe", name="Oe")
            nc.vector.tensor_copy(Oe[64:65, :], lr2[64:65, :])

            def epi_tail(h=h, ib=ib, lt=lt, Oe=Oe):
                Rp = pjps.tile([128, 512], F32, tag="pj", name="Rp")
                nc.tensor.matmul(Rp[0:128, :], onesc_sb[64:65, :],
                                 Oe[64:65, :], start=True, stop=True)
                nc.vector.tensor_tensor(Oe[0:64, :], lt[0:64, :],
                                        Rp[0:64, :], MULT)
                nc.vector.tensor_tensor(Oe[96:128, :], lt[96:128, :],
                                        Rp[96:128, :], MULT)
                for (sb, dlo, dlen) in ((0, 0, 64), (96, 64, 32)):
                    r = HS * h + dlo
                    s0, p0 = divmod(r, 128)
                    n0 = min(128 - p0, dlen)
                    nc.gpsimd.dma_start(oTp[p0:p0 + n0, s0, ib:ib + 512],
                                        Oe[sb:sb + n0, :])
                    if n0 < dlen:
                        nc.gpsimd.dma_start(
                            oTp[0:dlen - n0, s0 + 1, ib:ib + 512],
                            Oe[sb + n0:sb + dlen, :])

            epi_carry[0] = epi_tail

        def phase_c_tt(b, ta):
            tt = 4 * b + ta
            o_sb = ocp.tile([128, C], F32, tag="osb", name="osb")
            for (a, bb) in ((0, 512), (512, C)):
                cps = pjps.tile([128, 512], F32, tag="pj", name="cps")
                for s in range(KT):
                    nc.tensor.matmul(cps[:, 0:bb - a],
                                     oTp[:, s, tt * 128:(tt + 1) * 128],
                                     wpT[:, s, a:bb],
                                     start=(s == 0), stop=(s == KT - 1))
                nc.vector.tensor_tensor(o_sb[:, a:bb], cps[:, 0:bb - a],
                                        bo_sb[:, a:bb], ADD)
            nc.gpsimd.dma_start(out_r[:, tt, :], o_sb[:])

        def tw_items(s0):
            return [lambda s0=s0, kc=kc: (lambda t_, n:
                    tr_group(t_, range(n), kc,
                             waT[:, kc, s0 * 128:(s0 + n) * 128]))(*wch[s0])
                    for kc in range(KT)]

        def v_items(pr):
            items = [lambda: new_v()]
            items += [lambda pr=pr, tt=tt: v_proj(pr, (tt,))
                      for tt in range(TT)]
            return items

        def qk_stripe_items(j):
            return [lambda j=j, tc4=tc4: qk_chunk(j, tc4)
                    for tc4 in range(4)]

        def head(h, hh, feed):
            V_h = vtile[h // 2]
            qh, kh = qkt[h]
            for b in range(NB):
                attn_block(h, hh, b, qh, kh, V_h, feed)
            while feed:
                feed.pop(0)()

        def tx_items(c2):
            return [lambda c2=c2, kc=kc:
                    tr_group(xch[c2], range(2), kc,
                             xT[:, kc, c2 * 256:(c2 + 1) * 256])
                    for kc in range(KT)]

        def chunk_items(c4):
            """Everything head 0 needs for t-chunk c4 of its NEXT block."""
            items = tx_items(2 * c4)
            items += [lambda tt=tt: v_proj(0, (tt,))
                      for tt in range(4 * c4, 4 * c4 + 2)]
            items += tx_items(2 * c4 + 1)
            items += [lambda tt=tt: v_proj(0, (tt,))
                      for tt in range(4 * c4 + 2, 4 * c4 + 4)]
            items += [lambda j=j, c4=c4: qk_chunk(j, c4)
                      for j in (0, 6, 1, 7)]
            return items

        # ---------------- program ----------------
        # head 0, fused with phase-A x chunks; stripes 0,6,1,7 (q/k rows of
        # heads 0-2) projected per t-chunk; each block's phase-A work for the
        # NEXT t-chunk is fed into the current attention block
        new_qk(0)
        new_qk(1)
        new_qk(2)
        transpose_x(0)
        transpose_x(1)
        transpose_w(0)
        transpose_w(6)
        for j in (0, 6, 1, 7):
            qk_chunk(j, 0)
        transpose_w(12)   # v cols 0-255
        new_v()
        V0 = vtile[0]
        v_proj(0, range(0, 4))
        issue_w(2)        # q stripes 2,3
        issue_w(8)        # k stripes 8,9
        issue_w(14)       # v cols 256-511 (pairs 1,2)
        new_qk(3)
        h1_feed = (tw_items(2) + tw_items(8) + tw_items(14)
                   + v_items(1) + qk_stripe_items(2) + qk_stripe_items(8))
        for b in range(NB):
            fd = chunk_items(b + 1) if b < 3 else h1_feed
            attn_block(0, 0, b, qkt[0][0], qkt[0][1], V0, fd)
            if b < 3:
                while fd:
                    fd.pop(0)()
        head(1, 1, h1_feed)
        issue_w(4)        # q stripes 4,5
        issue_w(10)       # k stripes 10,11
        issue_w(16)       # v cols 512-767 (pairs 2,3)
        new_qk(4)
        new_qk(5)
        head(2, 0,
             qk_stripe_items(3) + qk_stripe_items(9)
             + tw_items(16) + tw_items(4))
        new_qk(6)
        head(3, 1,
             tw_items(10) + v_items(2)
             + qk_stripe_items(4) + qk_stripe_items(10))
        wpc = [pa_wp.tile([128, 3, C], F32R, tag="wpch", name=f"wpch{g}")
               for g in range(2)]
        nc.sync.dma_start(wpc[0][:], wp_r[:, 0:3, :])
        nc.sync.dma_start(wpc[1][:], wp_r[:, 3:6, :])
        new_qk(7)
        head(4, 0, qk_stripe_items(5) + qk_stripe_items(11))
        head(5, 1, v_items(3))
        wp_feed = [lambda g=g, kc=kc: tr_group(
                       wpc[g], range(3), kc, wpT[:, kc, g * 384:(g + 1) * 384])
                   for g in range(2) for kc in range(KT)]
        head(6, 0, wp_feed)
        # head 7: feed the output projection per completed i-block
        V7 = vtile[3]
        for b in range(NB):
            fd = ([lambda b=b, ta=ta: phase_c_tt(b - 1, ta)
                   for ta in range(4)] if b >= 1 else [])
            attn_block(7, 1, b, qkt[7][0], qkt[7][1], V7, fd,
                       early_flush=True)
            while fd:
                fd.pop(0)()
        flush_epi()
        for ta in range(4):
            phase_c_tt(3, ta)

    nc.finalize()
    return nc


_NC_CACHE = {}


def _get_nc():
    if "nc" not in _NC_CACHE:
        _NC_CACHE["nc"] = build_nc()
    return _NC_CACHE["nc"]


def _make_consts(b_attn, b_proj):
    s = 1.0 / math.sqrt(HS)
    qk_rows = np.concatenate([b_attn[0:C] * s, b_attn[C:2 * C]])
    bqkp = np.ascontiguousarray(
        qk_rows.reshape(12, 128).T.astype(np.float32))     # [128, 12]
    bv = np.ascontiguousarray(
        np.broadcast_to(b_attn[2 * C:3 * C], (128, C)).astype(np.float32))
    bo = np.ascontiguousarray(
        np.broadcast_to(b_proj, (128, C)).astype(np.float32))
    ident = np.eye(128, dtype=np.float32)
    mk = np.triu(np.ones((128, 128), dtype=np.float32)).astype(
        ml_dtypes.bfloat16)
    mkz = np.concatenate(
        [np.zeros((128, 128), dtype=np.float32),
         np.triu(np.ones((128, 128), dtype=np.float32))],
        axis=1).astype(ml_dtypes.bfloat16)
    onesc = np.ones((128, 128), dtype=np.float32).astype(ml_dtypes.bfloat16)
    return bqkp, bv, bo, ident, mk, mkz, onesc


def kernel(x, w_attn, b_attn, w_proj, b_proj, _want_results=False, **run_kwargs):
    x = np.asarray(x, dtype=np.float32)
    w_attn = np.asarray(w_attn, dtype=np.float32)
    b_attn = np.asarray(b_attn, dtype=np.float32)
    w_proj = np.asarray(w_proj, dtype=np.float32)
    b_proj = np.asarray(b_proj, dtype=np.float32)

    s = 1.0 / math.sqrt(HS)
    wat = w_attn.copy()
    wat[0:C, :] *= s            # fold the 1/sqrt(hs) logit scale into Q
    bqkp, bv, bo, ident, mk, mkz, onesc = _make_consts(b_attn, b_proj)

    nc = _get_nc()
    common = dict(wat=wat, wp=w_proj, ident=ident, mk=mk, mkz=mkz,
                  onesc=onesc, bqkp=bqkp, bv=bv, bo=bo)
    in_maps = [dict(x_b=np.ascontiguousarray(x[c]), **common)
               for c in range(NCORES)]
    res = run_bass_kernel_spmd(nc, in_maps, core_ids=list(range(NCORES)),
                               **run_kwargs)
    out = np.stack([res.results[c]["out"] for c in range(NCORES)], axis=0)
    if _want_results:
        return out, res
    return out


if __name__ == "__main__":
    rng = np.random.default_rng(0)
    x = rng.standard_normal((B, T, C), dtype=np.float32)
    w_attn = rng.standard_normal((3 * C, C), dtype=np.float32) / math.sqrt(C)
    b_attn = rng.standard_normal(3 * C).astype(np.float32) * 0.02
    w_proj = rng.standard_normal((C, C), dtype=np.float32) / math.sqrt(C)
    b_proj = rng.standard_normal(C).astype(np.float32) * 0.02
    o = kernel(x, w_attn, b_attn, w_proj, b_proj)
    print("out", o.shape, o.dtype, float(np.abs(o).mean()))


# revision 3
# speedup vs baseline: 1.0169x; 1.0169x over previous
"""Causal multi-head attention block (B=8, T=2048, C=768, H=8) on 8 trn2 cores.

Sharding: data-parallel over batch — one batch element per NeuronCore, weights
replicated, no collectives.

On top of v2 (all-bf16 SBUF-resident, no DRAM scratch, paired-j-tile attention
with fed-in projection work and a double-buffered softmax epilogue), v4 packs
the Q/K projections into 12 M=128 stripe groups (instead of 16 M=96 head
groups): the projection output is evicted into a packed staging tile and
repacked into head-aligned [96, T] q/k tiles by sb->sb DMA on the otherwise
idle sync queue. That removes a quarter of the projection matmul cycles and
96 weight-load serializations.

Phases per core:
  A: x / w_attn / w_proj PE-transposed into resident bf16 tiles, x in 256-row
     chunks fused into head 0's blocks.
  B: per head: causal attention in S^T layout over 512-wide i-blocks and
     j-tile pairs (one exp per pair; PV runs one pair behind its S/exp so the
     exp latency hides behind the next pair's S plus fed projection items;
     diagonal masks on Pool). The denominator accumulates via a ones column
     in PSUM row 64; the epilogue reciprocal (approx DVE op), PE broadcast,
     DVE normalize, and a sb->sb DMA pack into 128-row O^T stripes.
  C: out = O^T.T @ w_proj^T + b_proj over packed stripes, fed into head 7.
"""

import math
import os
import sys
from contextlib import ExitStack

for _p in ("/opt/trn_rl_repo", "/root/.axon_site/_ro/trn_rl_repo"):
    if os.path.isdir(_p) and _p not in sys.path:
        sys.path.append(_p)

import numpy as np
import ml_dtypes

import concourse.bass as bass  # noqa: F401
from concourse import bacc
import concourse.mybir as mybir
import concourse.tile as tile
from concourse.bass_utils import run_bass_kernel_spmd

F32 = mybir.dt.float32
F32R = mybir.dt.float32r
BF16 = mybir.dt.bfloat16
EXP = mybir.ActivationFunctionType.Exp
ADD = mybir.AluOpType.add
MULT = mybir.AluOpType.mult

B, T, C, H, HS = 8, 2048, 768, 8, 96
KT = C // 128         # 6 contraction tiles of 128
TT = T // 128         # 16 t-tiles of 128
NB = T // 512         # 4 i-blocks of 512
NCORES = 8


def build_nc():
    nc = bacc.Bacc()
    x_b = nc.dram_tensor("x_b", [T, C], F32R, kind="ExternalInput")
    wat = nc.dram_tensor("wat", [3 * C, C], F32R, kind="ExternalInput")
    wp = nc.dram_tensor("wp", [C, C], F32R, kind="ExternalInput")
    ident = nc.dram_tensor("ident", [128, 128], F32R, kind="ExternalInput")
    mk = nc.dram_tensor("mk", [128, 128], BF16, kind="ExternalInput")
    mkz = nc.dram_tensor("mkz", [128, 256], BF16, kind="ExternalInput")
    onesc = nc.dram_tensor("onesc", [128, 128], BF16, kind="ExternalInput")
    bqkp = nc.dram_tensor("bqkp", [128, 12], F32, kind="ExternalInput")
    bv = nc.dram_tensor("bv", [128, C], F32, kind="ExternalInput")
    bo = nc.dram_tensor("bo", [128, C], F32, kind="ExternalInput")
    out = nc.dram_tensor("out", [T, C], F32, kind="ExternalOutput")

    x_r = x_b.rearrange("(a p) c -> p a c", p=128)     # [128, 16, C]
    wat_r = wat.rearrange("(a p) c -> p a c", p=128)   # [128, 18, C]
    wp_r = wp.rearrange("(a p) c -> p a c", p=128)     # [128, 6, C]
    out_r = out.rearrange("(a p) c -> p a c", p=128)   # [128, 16, C]

    with tile.TileContext(nc) as tc, ExitStack() as ctx:
        # ---------------- pools ----------------
        consts = ctx.enter_context(tc.tile_pool(name="consts", bufs=1))
        id_sb = consts.tile([128, 128], F32R, tag="id")
        mk_sb = consts.tile([128, 128], BF16, tag="mk")
        mkz_sb = consts.tile([128, 256], BF16, tag="mkz")
        onesc_sb = consts.tile([128, 128], BF16, tag="onesc")
        bqkp_sb = consts.tile([128, 12], F32, tag="bqkp")
        bv_sb = consts.tile([128, C], F32, tag="bv")
        bo_sb = consts.tile([128, C], F32, tag="bo")
        one_bf = consts.tile([128, 1], BF16, tag="onebf")
        nc.vector.memset(one_bf[:], 1.0)

        persist = ctx.enter_context(tc.tile_pool(name="persist", bufs=1))
        xT = persist.tile([128, KT, T], BF16, tag="xT")        # x^T
        waT = persist.tile([128, KT, 3 * C], BF16, tag="waT")  # w_attn^T
        wpT = persist.tile([128, KT, C], BF16, tag="wpT")      # w_proj^T
        oTp = persist.tile([128, KT, T], BF16, tag="oTp")      # packed O^T

        pa_x = ctx.enter_context(tc.tile_pool(name="pa_x", bufs=2))
        pa_w = ctx.enter_context(tc.tile_pool(name="pa_w", bufs=3))
        pa_wp = ctx.enter_context(tc.tile_pool(name="pa_wp", bufs=2))
        vsbp = ctx.enter_context(tc.tile_pool(name="vsb", bufs=2))
        qkp = ctx.enter_context(tc.tile_pool(name="qk", bufs=6))
        stgp = ctx.enter_context(tc.tile_pool(name="stg", bufs=3))
        p2p = ctx.enter_context(tc.tile_pool(name="p2", bufs=3))
        epp = ctx.enter_context(tc.tile_pool(name="ep", bufs=2))
        ocp = ctx.enter_context(tc.tile_pool(name="oc", bufs=2))
        # PSUM: pj(2 bufs x 1 bank) + S-pair(2 bufs x 2 banks) + O(2 x 1) = 8
        pjps = ctx.enter_context(tc.tile_pool(name="pj", bufs=2, space="PSUM"))
        sps = ctx.enter_context(tc.tile_pool(name="sp", bufs=2, space="PSUM"))
        ops = ctx.enter_context(tc.tile_pool(name="op", bufs=2, space="PSUM"))

        # ---------------- DMA staging ----------------
        wch = {}

        def issue_w(s0, n=2):
            t_ = pa_w.tile([128, 2, C], F32R, tag="wch", name=f"w{s0}")
            nc.sync.dma_start(t_[:, 0:n, :], wat_r[:, s0:s0 + n, :])
            wch[s0] = (t_, n)

        nc.sync.dma_start(id_sb[:], ident[:, :])
        # x in 8 chunks of 2 t-tiles (first chunk first so transposes start
        # as early as possible)
        xch = [pa_x.tile([128, 2, C], F32R, tag="xch", name=f"xch{c2}")
               for c2 in range(8)]
        nc.sync.dma_start(xch[0][:], x_r[:, 0:2, :])
        nc.sync.dma_start(xch[1][:], x_r[:, 2:4, :])
        issue_w(0)       # q stripes 0,1 (heads 0-2)
        issue_w(6)       # k stripes 6,7 (heads 0-2)
        issue_w(12)      # v cols 0-255 (head pair 0)
        nc.sync.dma_start(mk_sb[:], mk[:, :])
        nc.sync.dma_start(mkz_sb[:], mkz[:, :])
        nc.sync.dma_start(onesc_sb[:], onesc[:, :])
        nc.sync.dma_start(bqkp_sb[:], bqkp[:, :])
        nc.sync.dma_start(bv_sb[:], bv[:, :])
        nc.sync.dma_start(bo_sb[:], bo[:, :])
        for c2 in range(2, 8):
            nc.sync.dma_start(xch[c2][:], x_r[:, 2 * c2:2 * c2 + 2, :])

        # ---------------- helpers ----------------
        def tr_group(src, idxs, kc, dst):
            n = len(idxs)
            psb = pjps.tile([128, 512], F32R, tag="pj", name="trps")
            for i, sl in enumerate(idxs):
                nc.tensor.matmul(psb[:, i * 128:(i + 1) * 128],
                                 src[:, sl, kc * 128:(kc + 1) * 128],
                                 id_sb[:], is_transpose=True,
                                 start=(i == 0), stop=(i == n - 1))
            nc.any.tensor_copy(dst, psb[:, 0:n * 128])

        def transpose_w(s0):
            t_, n = wch[s0]
            for kc in range(KT):
                tr_group(t_, range(n), kc,
                         waT[:, kc, s0 * 128:(s0 + n) * 128])

        def transpose_x(c2):
            for kc in range(KT):
                tr_group(xch[c2], range(2), kc,
                         xT[:, kc, c2 * 256:(c2 + 1) * 256])

        vtile = {}
        vstate = [None]

        def new_v():
            # V head columns at slots 0-63 / 96-127, ones at 64-95: the
            # denominator accumulates in PSUM row 64 and O in rows 0-63 /
            # 96-127 (>32-partition APs must start at partition 0; bases
            # 64/96 are legal for <=32-partition APs and 1-row operands).
            V = vsbp.tile([128, TT, 2, 128], BF16, tag="V", name="V")
            nc.vector.tensor_copy(
                V[:, :, :, 64:96],
                one_bf[:].to_broadcast([128, TT, 2, 32]))
            vstate[0] = V
            vtile[len(vtile)] = V

        def v_proj(pr, tts):
            V = vtile[pr]
            c0 = 2 * C + 2 * HS * pr
            for tt in tts:
                vps = pjps.tile([128, 512], F32, tag="pj", name="vps")
                for kc in range(KT):
                    nc.tensor.matmul(vps[:, 0:2 * HS],
                                     xT[:, kc, tt * 128:(tt + 1) * 128],
                                     waT[:, kc, c0:c0 + 2 * HS],
                                     start=(kc == 0), stop=(kc == KT - 1))
                vr = vps[:, 0:2 * HS].rearrange("p (h d) -> p h d", d=HS)
                br = (bv_sb[:, 2 * HS * pr:2 * HS * (pr + 1)]
                      .rearrange("p (h d) -> p h d", d=HS))
                nc.vector.tensor_tensor(V[:, tt, :, 0:64],
                                        vr[:, :, 0:64], br[:, :, 0:64], ADD)
                nc.vector.tensor_tensor(V[:, tt, :, 96:128],
                                        vr[:, :, 64:HS], br[:, :, 64:HS], ADD)

        # --- packed Q/K projection: stripe j = packed rows 128j..128j+127
        # of [q rows 0-767 | k rows 768-1535]; lhsT M=128 spans head
        # boundaries; repack into head-aligned q/k tiles via sync-queue DMA.
        qkt = {}     # head -> [qT, kT] tiles [96, T] bf16

        def new_qk(h):
            qkt[h] = [qkp.tile([HS, T], BF16, tag="qk", name=f"qk{h}_{i}")
                      for i in range(2)]

        def qk_chunk(j, tc4):
            """Stripe j projection for t-chunk tc4: 6-matmul chain, bias-add
            evict into a small staging tile, then sync-queue DMA repack into
            the head-aligned q/k tiles (a stripe spans two heads)."""
            st = stgp.tile([128, 512], BF16, tag="stg", name="stg")
            pj = pjps.tile([128, 512], F32, tag="pj", name="qkps")
            for kc in range(KT):
                nc.tensor.matmul(
                    pj[:, 0:512],
                    waT[:, kc, 128 * j:128 * (j + 1)],
                    xT[:, kc, tc4 * 512:(tc4 + 1) * 512],
                    start=(kc == 0), stop=(kc == KT - 1))
            nc.vector.tensor_tensor(
                st[:, :], pj[:, 0:512],
                bqkp_sb[:, j:j + 1].to_broadcast([128, 512]), ADD)
            m = 0 if j < 6 else 1
            rowbase = 128 * (j % 6)
            lo, hi = tc4 * 512, (tc4 + 1) * 512
            r = rowbase
            while r < rowbase + 128:
                hd = r // HS
                rend = min((hd + 1) * HS, rowbase + 128)
                nc.sync.dma_start(
                    qkt[hd][m][r - HS * hd:rend - HS * hd, lo:hi],
                    st[r - rowbase:rend - rowbase, :])
                r = rend

        epi_carry = [None]   # deferred epilogue tail of the previous block

        def flush_epi():
            if epi_carry[0] is not None:
                epi_carry[0]()
                epi_carry[0] = None

        def attn_block(h, hh, b, qT, kT_, V, feed=None, early_flush=False):
            """Causal attention for head h, i-block b; PV lags one pair; the
            epilogue's PE part (denominator broadcast) and normalize/pack run
            deferred, early in the NEXT block, so the PE never waits on the
            copy->reciprocal DVE chain at a block boundary."""
            ib = 512 * b
            O_ps = ops.tile([128, 512], F32, tag="O", name="Ops")
            npair = 2 * b + 2

            def emit_pv(P2, p, de):
                for e in range(2):
                    jt = 2 * p + e
                    nc.tensor.matmul(O_ps[0:128, de:512],
                                     V[:, jt, hh, :],
                                     P2[:, e, de:512],
                                     start=(p == 0 and e == 0),
                                     stop=(p == npair - 1 and e == 1))

            pend = []
            for p in range(npair):
                de = 0 if p <= 2 * b else 256
                S2 = sps.tile([128, 2, 512], F32, tag="S", name="S2")
                for e in range(2):
                    jt = 2 * p + e
                    nc.tensor.matmul(S2[:, e, de:512],
                                     kT_[0:HS, jt * 128:(jt + 1) * 128],
                                     qT[0:HS, ib + de:ib + 512],
                                     start=True, stop=True)
                P2 = p2p.tile([128, 2, 512], BF16, tag="P", name="P2")
                nc.scalar.activation(P2[:, :, de:512], S2[:, :, de:512], EXP)
                if p >= 2 * b:  # diagonal pair: causal masks
                    nc.gpsimd.tensor_tensor(P2[:, 0, de:de + 128],
                                            P2[:, 0, de:de + 128],
                                            mk_sb[:], MULT)
                    nc.gpsimd.tensor_tensor(P2[:, 1, de:de + 256],
                                            P2[:, 1, de:de + 256],
                                            mkz_sb[:], MULT)
                pend.append((P2, p, de))
                if p == 0 and early_flush:
                    flush_epi()
                if feed:
                    for _ in range(min(2, len(feed))):
                        feed.pop(0)()
                if p == 0:
                    flush_epi()
                if len(pend) > 2:
                    emit_pv(*pend.pop(0))
            while pend:
                emit_pv(*pend.pop(0))

            # epilogue, immediate DVE part
            lt = epp.tile([128, 512], F32, tag="lt", name="lt")
            nc.vector.tensor_copy(lt[:], O_ps[0:128, :])
            lr2 = epp.tile([65, 512], F32, tag="lr2", name="lr2", bufs=1)
            nc.vector.reciprocal_approx_fast(lr2[0:65, :], lt[0:65, :])
            Oe = epp.tile([128, 512], BF16, tag="Oe", name="Oe")
            nc.vector.tensor_copy(Oe[64:65, :], lr2[64:65, :])

            def epi_tail(h=h, ib=ib, lt=lt, Oe=Oe):
                Rp = pjps.tile([128, 512], F32, tag="pj", name="Rp")
                nc.tensor.matmul(Rp[0:128, :], onesc_sb[64:65, :],
                                 Oe[64:65, :], start=True, stop=True)
                nc.vector.tensor_tensor(Oe[0:64, :], lt[0:64, :],
                                        Rp[0:64, :], MULT)
                nc.vector.tensor_tensor(Oe[96:128, :], lt[96:128, :],
                                        Rp[96:128, :], MULT)
                for (sb, dlo, dlen) in ((0, 0, 64), (96, 64, 32)):
                    r = HS * h + dlo
                    s0, p0 = divmod(r, 128)
                    n0 = min(128 - p0, dlen)
                    nc.gpsimd.dma_start(oTp[p0:p0 + n0, s0, ib:ib + 512],
                                        Oe[sb:sb + n0, :])
                    if n0 < dlen:
                        nc.gpsimd.dma_start(
                            oTp[0:dlen - n0, s0 + 1, ib:ib + 512],
                            Oe[sb + n0:sb + dlen, :])

            epi_carry[0] = epi_tail

        def phase_c_tt(b, ta):
            tt = 4 * b + ta
            o_sb = ocp.tile([128, C], F32, tag="osb", name="osb")
            for (a, bb) in ((0, 512), (512, C)):
                cps = pjps.tile([128, 512], F32, tag="pj", name="cps")
                for s in range(KT):
                    nc.tensor.matmul(cps[:, 0:bb - a],
                                     oTp[:, s, tt * 128:(tt + 1) * 128],
                                     wpT[:, s, a:bb],
                                     start=(s == 0), stop=(s == KT - 1))
                nc.vector.tensor_tensor(o_sb[:, a:bb], cps[:, 0:bb - a],
                                        bo_sb[:, a:bb], ADD)
            nc.gpsimd.dma_start(out_r[:, tt, :], o_sb[:])

        def tw_items(s0):
            return [lambda s0=s0, kc=kc: (lambda t_, n:
                    tr_group(t_, range(n), kc,
                             waT[:, kc, s0 * 128:(s0 + n) * 128]))(*wch[s0])
                    for kc in range(KT)]

        def v_items(pr):
            items = [lambda: new_v()]
            items += [lambda pr=pr, tt=tt: v_proj(pr, (tt,))
                      for tt in range(TT)]
            return items

        def qk_stripe_items(j):
            return [lambda j=j, tc4=tc4: qk_chunk(j, tc4)
                    for tc4 in range(4)]

        def head(h, hh, feed):
            V_h = vtile[h // 2]
            qh, kh = qkt[h]
            for b in range(NB):
                attn_block(h, hh, b, qh, kh, V_h, feed)
            while feed:
                feed.pop(0)()

        def tx_items(c2):
            return [lambda c2=c2, kc=kc:
                    tr_group(xch[c2], range(2), kc,
                             xT[:, kc, c2 * 256:(c2 + 1) * 256])
                    for kc in range(KT)]

        def chunk_items(c4):
            """Everything head 0 needs for t-chunk c4 of its NEXT block."""
            items = tx_items(2 * c4)
            items += [lambda tt=tt: v_proj(0, (tt,))
                      for tt in range(4 * c4, 4 * c4 + 2)]
            items += tx_items(2 * c4 + 1)
            items += [lambda tt=tt: v_proj(0, (tt,))
                      for tt in range(4 * c4 + 2, 4 * c4 + 4)]
            items += [lambda j=j, c4=c4: qk_chunk(j, c4)
                      for j in (0, 6, 1, 7)]
            return items

        # ---------------- program ----------------
        # head 0, fused with phase-A x chunks; stripes 0,6,1,7 (q/k rows of
        # heads 0-2) projected per t-chunk; each block's phase-A work for the
        # NEXT t-chunk is fed into the current attention block
        new_qk(0)
        new_qk(1)
        new_qk(2)
        transpose_x(0)
        transpose_x(1)
        transpose_w(0)
        transpose_w(6)
        for j in (0, 6, 1, 7):
            qk_chunk(j, 0)
        transpose_w(12)   # v cols 0-255
        new_v()
        V0 = vtile[0]
        v_proj(0, range(0, 4))
        issue_w(2)        # q stripes 2,3
        issue_w(8)        # k stripes 8,9
        issue_w(14)       # v cols 256-511 (pairs 1,2)
        new_qk(3)
        h1_feed = (tw_items(2) + tw_items(8) + tw_items(14)
                   + v_items(1) + qk_stripe_items(2) + qk_stripe_items(8))
        for b in range(NB):
            fd = chunk_items(b + 1) if b < 3 else h1_feed
            attn_block(0, 0, b, qkt[0][0], qkt[0][1], V0, fd)
            if b < 3:
                while fd:
                    fd.pop(0)()
        head(1, 1, h1_feed)
        issue_w(4)        # q stripes 4,5
        issue_w(10)       # k stripes 10,11
        issue_w(16)       # v cols 512-767 (pairs 2,3)
        new_qk(4)
        new_qk(5)
        head(2, 0,
             qk_stripe_items(3) + qk_stripe_items(9)
             + tw_items(16) + tw_items(4))
        new_qk(6)
        head(3, 1,
             tw_items(10) + v_items(2)
             + qk_stripe_items(4) + qk_stripe_items(10))
        wpc = [pa_wp.tile([128, 3, C], F32R, tag="wpch", name=f"wpch{g}")
               for g in range(2)]
        nc.sync.dma_start(wpc[0][:], wp_r[:, 0:3, :])
        nc.sync.dma_start(wpc[1][:], wp_r[:, 3:6, :])
        new_qk(7)
        head(4, 0, qk_stripe_items(5) + qk_stripe_items(11))
        head(5, 1, v_items(3))
        wp_feed = [lambda g=g, kc=kc: tr_group(
                       wpc[g], range(3), kc, wpT[:, kc, g * 384:(g + 1) * 384])
                   for g in range(2) for kc in range(KT)]
        head(6, 0, wp_feed)
        # head 7: feed the output projection per completed i-block
        V7 = vtile[3]
        for b in range(NB):
            fd = ([lambda b=b, ta=ta: phase_c_tt(b - 1, ta)
                   for ta in range(4)] if b >= 1 else [])
            attn_block(7, 1, b, qkt[7][0], qkt[7][1], V7, fd,
                       early_flush=True)
            while fd:
                fd.pop(0)()
        flush_epi()
        for ta in range(4):
            phase_c_tt(3, ta)

    nc.finalize()
    return nc


_NC_CACHE = {}


def _get_nc():
    if "nc" not in _NC_CACHE:
        _NC_CACHE["nc"] = build_nc()
    return _NC_CACHE["nc"]


def _make_consts(b_attn, b_proj):
    s = 1.0 / math.sqrt(HS)
    qk_rows = np.concatenate([b_attn[0:C] * s, b_attn[C:2 * C]])
    bqkp = np.ascontiguousarray(
        qk_rows.reshape(12, 128).T.astype(np.float32))     # [128, 12]
    bv = np.ascontiguousarray(
        np.broadcast_to(b_attn[2 * C:3 * C], (128, C)).astype(np.float32))
    bo = np.ascontiguousarray(
        np.broadcast_to(b_proj, (128, C)).astype(np.float32))
    ident = np.eye(128, dtype=np.float32)
    mk = np.triu(np.ones((128, 128), dtype=np.float32)).astype(
        ml_dtypes.bfloat16)
    mkz = np.concatenate(
        [np.zeros((128, 128), dtype=np.float32),
         np.triu(np.ones((128, 128), dtype=np.float32))],
        axis=1).astype(ml_dtypes.bfloat16)
    onesc = np.ones((128, 128), dtype=np.float32).astype(ml_dtypes.bfloat16)
    return bqkp, bv, bo, ident, mk, mkz, onesc


def kernel(x, w_attn, b_attn, w_proj, b_proj, _want_results=False, **run_kwargs):
    x = np.asarray(x, dtype=np.float32)
    w_attn = np.asarray(w_attn, dtype=np.float32)
    b_attn = np.asarray(b_attn, dtype=np.float32)
    w_proj = np.asarray(w_proj, dtype=np.float32)
    b_proj = np.asarray(b_proj, dtype=np.float32)

    s = 1.0 / math.sqrt(HS)
    wat = w_attn.copy()
    wat[0:C, :] *= s            # fold the 1/sqrt(hs) logit scale into Q
    bqkp, bv, bo, ident, mk, mkz, onesc = _make_consts(b_attn, b_proj)

    nc = _get_nc()
    common = dict(wat=wat, wp=w_proj, ident=ident, mk=mk, mkz=mkz,
                  onesc=onesc, bqkp=bqkp, bv=bv, bo=bo)
    in_maps = [dict(x_b=np.ascontiguousarray(x[c]), **common)
               for c in range(NCORES)]
    res = run_bass_kernel_spmd(nc, in_maps, core_ids=list(range(NCORES)),
                               **run_kwargs)
    out = np.stack([res.results[c]["out"] for c in range(NCORES)], axis=0)
    if _want_results:
        return out, res
    return out


if __name__ == "__main__":
    rng = np.random.default_rng(0)
    x = rng.standard_normal((B, T, C), dtype=np.float32)
    w_attn = rng.standard_normal((3 * C, C), dtype=np.float32) / math.sqrt(C)
    b_attn = rng.standard_normal(3 * C).astype(np.float32) * 0.02
    w_proj = rng.standard_normal((C, C), dtype=np.float32) / math.sqrt(C)
    b_proj = rng.standard_normal(C).astype(np.float32) * 0.02
    o = kernel(x, w_attn, b_attn, w_proj, b_proj)
    print("out", o.shape, o.dtype, float(np.abs(o).mean()))


# revision 4
# speedup vs baseline: 1.0448x; 1.0274x over previous
"""Causal multi-head attention block (B=8, T=2048, C=768, H=8) on 8 trn2 cores.

Sharding: data-parallel over batch — one batch element per NeuronCore, weights
replicated, no collectives.

On top of v2 (all-bf16 SBUF-resident, no DRAM scratch, paired-j-tile attention
with fed-in projection work and a double-buffered softmax epilogue), v4 packs
the Q/K projections into 12 M=128 stripe groups (instead of 16 M=96 head
groups): the projection output is evicted into a packed staging tile and
repacked into head-aligned [96, T] q/k tiles by sb->sb DMA on the otherwise
idle sync queue. That removes a quarter of the projection matmul cycles and
96 weight-load serializations.

Phases per core:
  A: x / w_attn / w_proj PE-transposed into resident bf16 tiles, x in 256-row
     chunks fused into head 0's blocks.
  B: per head: causal attention in S^T layout over 512-wide i-blocks and
     j-tile pairs (one exp per pair; PV runs one pair behind its S/exp so the
     exp latency hides behind the next pair's S plus fed projection items;
     diagonal masks on Pool). The denominator accumulates via a ones column
     in PSUM row 64; the epilogue reciprocal (approx DVE op), PE broadcast,
     DVE normalize, and a sb->sb DMA pack into 128-row O^T stripes.
  C: out = O^T.T @ w_proj^T + b_proj over packed stripes, fed into head 7.
"""

import math
import os
import sys
from contextlib import ExitStack

for _p in ("/opt/trn_rl_repo", "/root/.axon_site/_ro/trn_rl_repo"):
    if os.path.isdir(_p) and _p not in sys.path:
        sys.path.append(_p)

import numpy as np
import ml_dtypes

import concourse.bass as bass  # noqa: F401
from concourse import bacc
import concourse.mybir as mybir
import concourse.tile as tile
from concourse.bass_utils import run_bass_kernel_spmd

F32 = mybir.dt.float32
F32R = mybir.dt.float32r
BF16 = mybir.dt.bfloat16
EXP = mybir.ActivationFunctionType.Exp
ADD = mybir.AluOpType.add
MULT = mybir.AluOpType.mult

B, T, C, H, HS = 8, 2048, 768, 8, 96
KT = C // 128         # 6 contraction tiles of 128
TT = T // 128         # 16 t-tiles of 128
NB = T // 512         # 4 i-blocks of 512
NCORES = 8


def build_nc():
    nc = bacc.Bacc()
    x_b = nc.dram_tensor("x_b", [T, C], F32R, kind="ExternalInput")
    wat = nc.dram_tensor("wat", [3 * C, C], F32R, kind="ExternalInput")
    wp = nc.dram_tensor("wp", [C, C], F32R, kind="ExternalInput")
    ident = nc.dram_tensor("ident", [128, 128], F32R, kind="ExternalInput")
    mk = nc.dram_tensor("mk", [128, 128], BF16, kind="ExternalInput")
    mkz = nc.dram_tensor("mkz", [128, 256], BF16, kind="ExternalInput")
    onesc = nc.dram_tensor("onesc", [128, 128], BF16, kind="ExternalInput")
    bqkp = nc.dram_tensor("bqkp", [128, 12], F32, kind="ExternalInput")
    bv = nc.dram_tensor("bv", [128, C], F32, kind="ExternalInput")
    bo = nc.dram_tensor("bo", [128, C], F32, kind="ExternalInput")
    out = nc.dram_tensor("out", [T, C], F32, kind="ExternalOutput")

    x_r = x_b.rearrange("(a p) c -> p a c", p=128)     # [128, 16, C]
    wat_r = wat.rearrange("(a p) c -> p a c", p=128)   # [128, 18, C]
    wp_r = wp.rearrange("(a p) c -> p a c", p=128)     # [128, 6, C]
    out_r = out.rearrange("(a p) c -> p a c", p=128)   # [128, 16, C]

    with tile.TileContext(nc) as tc, ExitStack() as ctx:
        # ---------------- pools ----------------
        consts = ctx.enter_context(tc.tile_pool(name="consts", bufs=1))
        id_sb = consts.tile([128, 128], F32R, tag="id")
        mk_sb = consts.tile([128, 128], BF16, tag="mk")
        mkz_sb = consts.tile([128, 256], BF16, tag="mkz")
        onesc_sb = consts.tile([128, 128], BF16, tag="onesc")
        bqkp_sb = consts.tile([128, 12], F32, tag="bqkp")
        bv_sb = consts.tile([128, C], F32, tag="bv")
        bo_sb = consts.tile([128, C], F32, tag="bo")
        one_bf = consts.tile([128, 1], BF16, tag="onebf")
        nc.vector.memset(one_bf[:], 1.0)

        persist = ctx.enter_context(tc.tile_pool(name="persist", bufs=1))
        xT = persist.tile([128, KT, T], BF16, tag="xT")        # x^T
        waT = persist.tile([128, KT, 3 * C], BF16, tag="waT")  # w_attn^T
        wpT = persist.tile([128, KT, C], BF16, tag="wpT")      # w_proj^T
        oTp = persist.tile([128, KT, T], BF16, tag="oTp")      # packed O^T

        pa_x = ctx.enter_context(tc.tile_pool(name="pa_x", bufs=2))
        pa_w = ctx.enter_context(tc.tile_pool(name="pa_w", bufs=3))
        pa_wp = ctx.enter_context(tc.tile_pool(name="pa_wp", bufs=2))
        vsbp = ctx.enter_context(tc.tile_pool(name="vsb", bufs=2))
        qkp = ctx.enter_context(tc.tile_pool(name="qk", bufs=6))
        stgp = ctx.enter_context(tc.tile_pool(name="stg", bufs=3))
        p2p = ctx.enter_context(tc.tile_pool(name="p2", bufs=3))
        epp = ctx.enter_context(tc.tile_pool(name="ep", bufs=2))
        ocp = ctx.enter_context(tc.tile_pool(name="oc", bufs=2))
        # PSUM: pj(2 bufs x 1 bank) + S-pair(2 bufs x 2 banks) + O(2 x 1) = 8
        pjps = ctx.enter_context(tc.tile_pool(name="pj", bufs=2, space="PSUM"))
        sps = ctx.enter_context(tc.tile_pool(name="sp", bufs=2, space="PSUM"))
        ops = ctx.enter_context(tc.tile_pool(name="op", bufs=2, space="PSUM"))

        # ---------------- DMA staging ----------------
        wch = {}

        def issue_w(s0, n=2):
            t_ = pa_w.tile([128, 2, C], F32R, tag="wch", name=f"w{s0}")
            nc.sync.dma_start(t_[:, 0:n, :], wat_r[:, s0:s0 + n, :])
            wch[s0] = (t_, n)

        nc.sync.dma_start(id_sb[:], ident[:, :])
        # x in 8 chunks of 2 t-tiles (first chunk first so transposes start
        # as early as possible)
        xch = [pa_x.tile([128, 2, C], F32R, tag="xch", name=f"xch{c2}")
               for c2 in range(8)]
        nc.sync.dma_start(xch[0][:], x_r[:, 0:2, :])
        nc.sync.dma_start(xch[1][:], x_r[:, 2:4, :])
        issue_w(0)       # q stripes 0,1 (heads 0-2)
        issue_w(6)       # k stripes 6,7 (heads 0-2)
        issue_w(12)      # v cols 0-255 (head pair 0)
        nc.sync.dma_start(mk_sb[:], mk[:, :])
        nc.sync.dma_start(mkz_sb[:], mkz[:, :])
        nc.sync.dma_start(onesc_sb[:], onesc[:, :])
        nc.sync.dma_start(bqkp_sb[:], bqkp[:, :])
        nc.sync.dma_start(bv_sb[:], bv[:, :])
        nc.sync.dma_start(bo_sb[:], bo[:, :])
        for c2 in range(2, 8):
            nc.sync.dma_start(xch[c2][:], x_r[:, 2 * c2:2 * c2 + 2, :])

        # ---------------- helpers ----------------
        def tr_group(src, idxs, kc, dst):
            n = len(idxs)
            psb = pjps.tile([128, 512], F32R, tag="pj", name="trps")
            for i, sl in enumerate(idxs):
                nc.tensor.matmul(psb[:, i * 128:(i + 1) * 128],
                                 src[:, sl, kc * 128:(kc + 1) * 128],
                                 id_sb[:], is_transpose=True,
                                 start=(i == 0), stop=(i == n - 1))
            nc.any.tensor_copy(dst, psb[:, 0:n * 128])

        def transpose_w(s0):
            t_, n = wch[s0]
            for kc in range(KT):
                tr_group(t_, range(n), kc,
                         waT[:, kc, s0 * 128:(s0 + n) * 128])

        def transpose_x(c2):
            for kc in range(KT):
                tr_group(xch[c2], range(2), kc,
                         xT[:, kc, c2 * 256:(c2 + 1) * 256])

        vtile = {}
        vstate = [None]

        def new_v():
            # V head columns at slots 0-63 / 96-127, ones at 64-95: the
            # denominator accumulates in PSUM row 64 and O in rows 0-63 /
            # 96-127 (>32-partition APs must start at partition 0; bases
            # 64/96 are legal for <=32-partition APs and 1-row operands).
            V = vsbp.tile([128, TT, 2, 128], BF16, tag="V", name="V")
            nc.vector.tensor_copy(
                V[:, :, :, 64:96],
                one_bf[:].to_broadcast([128, TT, 2, 32]))
            vstate[0] = V
            vtile[len(vtile)] = V

        def v_proj(pr, tts):
            V = vtile[pr]
            c0 = 2 * C + 2 * HS * pr
            for tt in tts:
                vps = pjps.tile([128, 512], F32, tag="pj", name="vps")
                for kc in range(KT):
                    nc.tensor.matmul(vps[:, 0:2 * HS],
                                     xT[:, kc, tt * 128:(tt + 1) * 128],
                                     waT[:, kc, c0:c0 + 2 * HS],
                                     start=(kc == 0), stop=(kc == KT - 1))
                vr = vps[:, 0:2 * HS].rearrange("p (h d) -> p h d", d=HS)
                br = (bv_sb[:, 2 * HS * pr:2 * HS * (pr + 1)]
                      .rearrange("p (h d) -> p h d", d=HS))
                nc.vector.tensor_tensor(V[:, tt, :, 0:64],
                                        vr[:, :, 0:64], br[:, :, 0:64], ADD)
                nc.vector.tensor_tensor(V[:, tt, :, 96:128],
                                        vr[:, :, 64:HS], br[:, :, 64:HS], ADD)

        # --- packed Q/K projection: stripe j = packed rows 128j..128j+127
        # of [q rows 0-767 | k rows 768-1535]; lhsT M=128 spans head
        # boundaries; repack into head-aligned q/k tiles via sync-queue DMA.
        qkt = {}     # head -> [qT, kT] tiles [96, T] bf16

        def new_qk(h):
            qkt[h] = [qkp.tile([HS, T], BF16, tag="qk", name=f"qk{h}_{i}")
                      for i in range(2)]

        def qk_chunk(j, tc4):
            """Stripe j projection for t-chunk tc4: 6-matmul chain, bias-add
            evict into a small staging tile, then sync-queue DMA repack into
            the head-aligned q/k tiles (a stripe spans two heads)."""
            st = stgp.tile([128, 512], BF16, tag="stg", name="stg")
            pj = pjps.tile([128, 512], F32, tag="pj", name="qkps")
            for kc in range(KT):
                nc.tensor.matmul(
                    pj[:, 0:512],
                    waT[:, kc, 128 * j:128 * (j + 1)],
                    xT[:, kc, tc4 * 512:(tc4 + 1) * 512],
                    start=(kc == 0), stop=(kc == KT - 1))
            nc.vector.tensor_tensor(
                st[:, :], pj[:, 0:512],
                bqkp_sb[:, j:j + 1].to_broadcast([128, 512]), ADD)
            m = 0 if j < 6 else 1
            rowbase = 128 * (j % 6)
            lo, hi = tc4 * 512, (tc4 + 1) * 512
            r = rowbase
            while r < rowbase + 128:
                hd = r // HS
                rend = min((hd + 1) * HS, rowbase + 128)
                nc.sync.dma_start(
                    qkt[hd][m][r - HS * hd:rend - HS * hd, lo:hi],
                    st[r - rowbase:rend - rowbase, :])
                r = rend

        epi_carry = [None]   # deferred epilogue tail of the previous block

        def flush_epi():
            if epi_carry[0] is not None:
                epi_carry[0]()
                epi_carry[0] = None

        def attn_block(h, hh, b, qT, kT_, V, feed=None, early_flush=False):
            """Causal attention for head h, i-block b; PV lags one pair; the
            epilogue's PE part (denominator broadcast) and normalize/pack run
            deferred, early in the NEXT block, so the PE never waits on the
            copy->reciprocal DVE chain at a block boundary."""
            ib = 512 * b
            O_ps = ops.tile([128, 512], F32, tag="O", name="Ops")
            npair = 2 * b + 2

            def emit_pv(P2, p, de):
                for e in range(2):
                    jt = 2 * p + e
                    nc.tensor.matmul(O_ps[0:128, de:512],
                                     V[:, jt, hh, :],
                                     P2[:, e, de:512],
                                     start=(p == 0 and e == 0),
                                     stop=(p == npair - 1 and e == 1))

            pend = []
            for p in range(npair):
                de = 0 if p <= 2 * b else 256
                S2 = sps.tile([128, 2, 512], F32, tag="S", name="S2")
                for e in range(2):
                    jt = 2 * p + e
                    nc.tensor.matmul(S2[:, e, de:512],
                                     kT_[0:HS, jt * 128:(jt + 1) * 128],
                                     qT[0:HS, ib + de:ib + 512],
                                     start=True, stop=True)
                P2 = p2p.tile([128, 2, 512], BF16, tag="P", name="P2")
                nc.scalar.activation(P2[:, :, de:512], S2[:, :, de:512], EXP)
                if p >= 2 * b:  # diagonal pair: causal masks
                    nc.gpsimd.tensor_tensor(P2[:, 0, de:de + 128],
                                            P2[:, 0, de:de + 128],
                                            mk_sb[:], MULT)
                    nc.gpsimd.tensor_tensor(P2[:, 1, de:de + 256],
                                            P2[:, 1, de:de + 256],
                                            mkz_sb[:], MULT)
                pend.append((P2, p, de))
                if p == 0 and early_flush:
                    flush_epi()
                if feed:
                    for _ in range(min(2, len(feed))):
                        feed.pop(0)()
                if p == 0:
                    flush_epi()
                if len(pend) > 2:
                    emit_pv(*pend.pop(0))
            while pend:
                emit_pv(*pend.pop(0))

            # epilogue, immediate DVE part
            lt = epp.tile([128, 512], F32, tag="lt", name="lt")
            nc.vector.tensor_copy(lt[:], O_ps[0:128, :])
            lr2 = epp.tile([65, 512], F32, tag="lr2", name="lr2", bufs=1)
            nc.vector.reciprocal_approx_fast(lr2[0:65, :], lt[0:65, :])
            Oe = epp.tile([128, 512], BF16, tag="Oe", name="Oe")
            nc.vector.tensor_copy(Oe[64:65, :], lr2[64:65, :])

            def epi_tail(h=h, ib=ib, lt=lt, Oe=Oe):
                Rp = pjps.tile([128, 512], F32, tag="pj", name="Rp")
                nc.tensor.matmul(Rp[0:128, :], onesc_sb[64:65, :],
                                 Oe[64:65, :], start=True, stop=True)
                nc.vector.tensor_tensor(Oe[0:64, :], lt[0:64, :],
                                        Rp[0:64, :], MULT)
                nc.vector.tensor_tensor(Oe[96:128, :], lt[96:128, :],
                                        Rp[96:128, :], MULT)
                for (sb, dlo, dlen) in ((0, 0, 64), (96, 64, 32)):
                    r = HS * h + dlo
                    s0, p0 = divmod(r, 128)
                    n0 = min(128 - p0, dlen)
                    nc.gpsimd.dma_start(oTp[p0:p0 + n0, s0, ib:ib + 512],
                                        Oe[sb:sb + n0, :])
                    if n0 < dlen:
                        nc.gpsimd.dma_start(
                            oTp[0:dlen - n0, s0 + 1, ib:ib + 512],
                            Oe[sb + n0:sb + dlen, :])

            epi_carry[0] = epi_tail

        def phase_c_tt(b, ta):
            tt = 4 * b + ta
            o_sb = ocp.tile([128, C], F32, tag="osb", name="osb")
            for (a, bb) in ((0, 512), (512, C)):
                cps = pjps.tile([128, 512], F32, tag="pj", name="cps")
                for s in range(KT):
                    nc.tensor.matmul(cps[:, 0:bb - a],
                                     oTp[:, s, tt * 128:(tt + 1) * 128],
                                     wpT[:, s, a:bb],
                                     start=(s == 0), stop=(s == KT - 1))
                nc.vector.tensor_tensor(o_sb[:, a:bb], cps[:, 0:bb - a],
                                        bo_sb[:, a:bb], ADD)
            nc.gpsimd.dma_start(out_r[:, tt, :], o_sb[:])

        def tw_items(s0):
            return [lambda s0=s0, kc=kc: (lambda t_, n:
                    tr_group(t_, range(n), kc,
                             waT[:, kc, s0 * 128:(s0 + n) * 128]))(*wch[s0])
                    for kc in range(KT)]

        def v_items(pr):
            items = [lambda: new_v()]
            items += [lambda pr=pr, tt=tt: v_proj(pr, (tt,))
                      for tt in range(TT)]
            return items

        def qk_stripe_items(j):
            return [lambda j=j, tc4=tc4: qk_chunk(j, tc4)
                    for tc4 in range(4)]

        def head(h, hh, feed):
            V_h = vtile[h // 2]
            qh, kh = qkt[h]
            for b in range(NB):
                attn_block(h, hh, b, qh, kh, V_h, feed)
            while feed:
                feed.pop(0)()

        def tx_items(c2):
            return [lambda c2=c2, kc=kc:
                    tr_group(xch[c2], range(2), kc,
                             xT[:, kc, c2 * 256:(c2 + 1) * 256])
                    for kc in range(KT)]

        def chunk_items(c4):
            """Everything head 0 needs for t-chunk c4 of its NEXT block."""
            items = tx_items(2 * c4)
            items += [lambda tt=tt: v_proj(0, (tt,))
                      for tt in range(4 * c4, 4 * c4 + 2)]
            items += tx_items(2 * c4 + 1)
            items += [lambda tt=tt: v_proj(0, (tt,))
                      for tt in range(4 * c4 + 2, 4 * c4 + 4)]
            items += [lambda j=j, c4=c4: qk_chunk(j, c4)
                      for j in (0, 6, 1, 7)]
            return items

        # ---------------- program ----------------
        # head 0, fused with phase-A x chunks; stripes 0,6,1,7 (q/k rows of
        # heads 0-2) projected per t-chunk; each block's phase-A work for the
        # NEXT t-chunk is fed into the current attention block
        new_qk(0)
        new_qk(1)
        new_qk(2)
        transpose_x(0)
        transpose_x(1)
        transpose_w(0)
        transpose_w(6)
        for j in (0, 6, 1, 7):
            qk_chunk(j, 0)
        transpose_w(12)   # v cols 0-255
        new_v()
        V0 = vtile[0]
        v_proj(0, range(0, 4))
        issue_w(2)        # q stripes 2,3
        issue_w(8)        # k stripes 8,9
        issue_w(14)       # v cols 256-511 (pairs 1,2)
        new_qk(3)
        h1_feed = (tw_items(2) + qk_stripe_items(2)
                   + tw_items(8) + qk_stripe_items(8)
                   + tw_items(14) + v_items(1))
        for b in range(NB):
            fd = chunk_items(b + 1) if b < 3 else h1_feed
            attn_block(0, 0, b, qkt[0][0], qkt[0][1], V0, fd)
            if b < 3:
                while fd:
                    fd.pop(0)()
        head(1, 1, h1_feed)
        issue_w(4)        # q stripes 4,5
        issue_w(10)       # k stripes 10,11
        issue_w(16)       # v cols 512-767 (pairs 2,3)
        new_qk(4)
        new_qk(5)
        head(2, 0,
             qk_stripe_items(3) + qk_stripe_items(9)
             + tw_items(16) + tw_items(4) + tw_items(10))
        new_qk(6)
        head(3, 1,
             qk_stripe_items(4) + qk_stripe_items(10) + v_items(2))
        wpc = [pa_wp.tile([128, 3, C], F32R, tag="wpch", name=f"wpch{g}")
               for g in range(2)]
        nc.sync.dma_start(wpc[0][:], wp_r[:, 0:3, :])
        nc.sync.dma_start(wpc[1][:], wp_r[:, 3:6, :])
        new_qk(7)
        head(4, 0, qk_stripe_items(5) + qk_stripe_items(11))
        head(5, 1, v_items(3))
        wp_feed = [lambda g=g, kc=kc: tr_group(
                       wpc[g], range(3), kc, wpT[:, kc, g * 384:(g + 1) * 384])
                   for g in range(2) for kc in range(KT)]
        head(6, 0, wp_feed)
        # head 7: feed the output projection per completed i-block
        V7 = vtile[3]
        for b in range(NB):
            fd = ([lambda b=b, ta=ta: phase_c_tt(b - 1, ta)
                   for ta in range(4)] if b >= 1 else [])
            attn_block(7, 1, b, qkt[7][0], qkt[7][1], V7, fd,
                       early_flush=True)
            while fd:
                fd.pop(0)()
        flush_epi()
        for ta in range(4):
            phase_c_tt(3, ta)

    nc.finalize()
    return nc


_NC_CACHE = {}


def _get_nc():
    if "nc" not in _NC_CACHE:
        _NC_CACHE["nc"] = build_nc()
    return _NC_CACHE["nc"]


def _make_consts(b_attn, b_proj):
    s = 1.0 / math.sqrt(HS)
    qk_rows = np.concatenate([b_attn[0:C] * s, b_attn[C:2 * C]])
    bqkp = np.ascontiguousarray(
        qk_rows.reshape(12, 128).T.astype(np.float32))     # [128, 12]
    bv = np.ascontiguousarray(
        np.broadcast_to(b_attn[2 * C:3 * C], (128, C)).astype(np.float32))
    bo = np.ascontiguousarray(
        np.broadcast_to(b_proj, (128, C)).astype(np.float32))
    ident = np.eye(128, dtype=np.float32)
    mk = np.triu(np.ones((128, 128), dtype=np.float32)).astype(
        ml_dtypes.bfloat16)
    mkz = np.concatenate(
        [np.zeros((128, 128), dtype=np.float32),
         np.triu(np.ones((128, 128), dtype=np.float32))],
        axis=1).astype(ml_dtypes.bfloat16)
    onesc = np.ones((128, 128), dtype=np.float32).astype(ml_dtypes.bfloat16)
    return bqkp, bv, bo, ident, mk, mkz, onesc


def kernel(x, w_attn, b_attn, w_proj, b_proj, _want_results=False, **run_kwargs):
    x = np.asarray(x, dtype=np.float32)
    w_attn = np.asarray(w_attn, dtype=np.float32)
    b_attn = np.asarray(b_attn, dtype=np.float32)
    w_proj = np.asarray(w_proj, dtype=np.float32)
    b_proj = np.asarray(b_proj, dtype=np.float32)

    s = 1.0 / math.sqrt(HS)
    wat = w_attn.copy()
    wat[0:C, :] *= s            # fold the 1/sqrt(hs) logit scale into Q
    bqkp, bv, bo, ident, mk, mkz, onesc = _make_consts(b_attn, b_proj)

    nc = _get_nc()
    common = dict(wat=wat, wp=w_proj, ident=ident, mk=mk, mkz=mkz,
                  onesc=onesc, bqkp=bqkp, bv=bv, bo=bo)
    in_maps = [dict(x_b=np.ascontiguousarray(x[c]), **common)
               for c in range(NCORES)]
    res = run_bass_kernel_spmd(nc, in_maps, core_ids=list(range(NCORES)),
                               **run_kwargs)
    out = np.stack([res.results[c]["out"] for c in range(NCORES)], axis=0)
    if _want_results:
        return out, res
    return out


if __name__ == "__main__":
    rng = np.random.default_rng(0)
    x = rng.standard_normal((B, T, C), dtype=np.float32)
    w_attn = rng.standard_normal((3 * C, C), dtype=np.float32) / math.sqrt(C)
    b_attn = rng.standard_normal(3 * C).astype(np.float32) * 0.02
    w_proj = rng.standard_normal((C, C), dtype=np.float32) / math.sqrt(C)
    b_proj = rng.standard_normal(C).astype(np.float32) * 0.02
    o = kernel(x, w_attn, b_attn, w_proj, b_proj)
    print("out", o.shape, o.dtype, float(np.abs(o).mean()))
